# revision 124
# baseline (speedup 1.0000x reference)
"""Causal attention kernel for Trainium2, 8 NeuronCores, sequence-parallel.

Reference computation (T=4096, D=1024, fp32):
    q = x @ Wqk; logits = q @ x.T (causal masked); attn = softmax(logits)
    out = (attn @ x) @ Wov

Causal load balancing under one SPMD program: the 32 query row-tiles of 128
are assigned to cores as {c, 15-c, 16+c, 31-c} and host-permuted into 4
local "slots" ordered by visibility class. Slot m processes a fixed key
budget of 8*(m+1) key-tiles (keys in natural order, prefix [0, 1024*(m+1))),
which covers every core's visible range in that class. Causality inside the
budget is enforced by a host-provided additive mask (0 / -60000) that also
carries the diagonal triangle, so the program is core-independent while
skipping 37.5% of the score/AV matmul work.

Matmul precision: fp16 inputs (x, Wqk, Wov, attn) with fp32 PSUM
accumulation; q and o1 kept in fp16 on-chip. Softmax row max subtracted in
fp32; attn stored fp16 for the DMA-xbar transposes and AV.

Scheduling notes: input DMAs are issued in consumption order (xqt/wqk for
phase A first, then keys/masks); phase B runs slots largest-first so the
exp/transpose pipeline drains during B and phase E can start right after;
tiles are split per dependency unit (per-kg keys, per-chunk attn-transpose,
per-d o1) to keep cross-engine waits granular.
"""

import sys

sys.path.insert(0, "/opt/trn_rl_repo")

import numpy as np

import concourse.tile as tile
from concourse import bacc, mybir
from concourse.bass_utils import run_bass_kernel_spmd

T = 4096
D = 1024
NCORES = 8
RQ = T // NCORES  # 512 query rows per core
KC = D // 128  # 8 contraction chunks
NEG16 = -57344.0  # exactly representable in fp8e5m2

BKT = [8, 16, 24, 32]  # key tiles (128) processed per slot
BG = [b // 4 for b in BKT]  # 512-wide key groups per slot
OFFK = [0, 1024, 3072, 6144]  # slot column offsets in ragged score layout
STOT = 10240  # total score/mask columns
MPOFF = [0, 2, 6, 12]  # mpart offsets (prefix of BG)
NCH = [b // 8 for b in BKT]  # 1024-wide exp chunks per slot: 1,2,3,4
LQOFF = [0, 1, 3, 6]  # lq offsets (prefix of NCH)

f32 = mybir.dt.float32
f16 = mybir.dt.float16
f8 = mybir.dt.float8e5
f8e4 = mybir.dt.float8e4
DR = mybir.MatmulPerfMode.DoubleRow


def _build_nc():
    nc = bacc.Bacc(
        "TRN2", target_bir_lowering=False, debug=False, num_devices=NCORES
    )

    # B's moving operand in fp8 hi/lo with contraction-paired rows: dram
    # row 256g + 128i + p holds contraction index 256g + 2p + i (i is the
    # DoubleRow pair element); hi block then lo block per 1024-key column
    # group. Phase A stays fp16 (on the CENTERED Wqk, see below).
    xqt_d = nc.dram_tensor("xqt", [D, RQ], f16, kind="ExternalInput").ap()
    xtp8_d = nc.dram_tensor("xtp8", [2 * D, T], f8e4, kind="ExternalInput").ap()
    # rank-1 restore operands: Wqk is centered host-side (W' = Wqk - 0.5)
    # to halve fp8 reconstruction error; the dropped 0.5*rowsum(x) outer
    # product is restored by one extra DoubleRow matmul per score group
    # using a triple e4m3 split of rowsum(x) (6 cross products, 3 of the
    # 128 aug partitions used, rest zero).
    saug_q_d = nc.dram_tensor("saug_q", [3, 2 * RQ], f8e4, kind="ExternalInput").ap()
    saug_k_d = nc.dram_tensor("saug_k", [3, 2 * T], f8e4, kind="ExternalInput").ap()
    # x rows in fp8 hi/lo, host-permuted block-major: dram row
    # 2048c + 1024hl + 256pg + 128i + p holds hi/lo of logical row
    # 1024c + 256pg + 2p + i, so each block is one contiguous transfer and
    # partition p carries rows (2p, 2p+1) as DoubleRow contraction pairs
    xp8_d = nc.dram_tensor("xp8", [2 * T, D], f8e4, kind="ExternalInput").ap()
    # wqk (centered, fp16) columns are host-permuted so psA chunk mtd
    # yields qT rows d = 256*(mtd//2) + 2p + (mtd%2): partition p of the
    # evacuated q-hi/lo then carries the DoubleRow pair (2p, 2p+1) for B
    wqk_d = nc.dram_tensor("wqk", [D, D], f16, kind="ExternalInput").ap()
    wov_d = nc.dram_tensor("wov", [D, D], f16, kind="ExternalInput").ap()
    ident_d = nc.dram_tensor("ident", [128, 128], f16, kind="ExternalInput").ap()
    # causal mask, only the last 1024 keys of each slot's budget: for class
    # m (tiles 8m..8m+7) keys below 1024m are visible to every member core,
    # so only the final window carries the triangle / -inf region
    mask_d = nc.dram_tensor("mask", [128, 4096], f8, kind="ExternalInput").ap()
    out_d = nc.dram_tensor("out", [RQ, D], f16, kind="ExternalOutput").ap()

    with tile.TileContext(nc) as tc:
        # stack allocator: long-lived pools first
        consts = tc.alloc_tile_pool(name="consts", bufs=1)
        pt_pool = tc.alloc_tile_pool(name="ptpool", bufs=1)
        xpstream = tc.alloc_tile_pool(name="xpstream", bufs=4)
        xplate = tc.alloc_tile_pool(name="xplate", bufs=2)
        p_pool = tc.alloc_tile_pool(name="ppool", bufs=2)
        s_pool = tc.alloc_tile_pool(name="spool", bufs=2)
        qt_pool = tc.alloc_tile_pool(name="qt", bufs=1)
        xtp_pool = tc.alloc_tile_pool(name="xtpp", bufs=1)
        mask_pool = tc.alloc_tile_pool(name="maskp", bufs=1)
        wqk_pool = tc.alloc_tile_pool(name="wqkp", bufs=1)
        xqt_pool = tc.alloc_tile_pool(name="xqtp", bufs=1)

        # stats scratch: negmax 0:4, lsum 4:8, recip 8:12, mpart 12:32
        smalls = consts.tile([128, 32], f32, name="smalls")
        dum = consts.tile([128, 240], f16, name="dum")
        ident = consts.tile([128, 128], f16, name="ident")
        negmax = smalls[:, 0:4]
        lsum = smalls[:, 4:8]
        recip = smalls[:, 8:12]
        mpart = smalls[:, 12:32]

        # transposed attn, fp8, packed in key-PAIRS: the xbar moves 2-byte
        # granules, so attn fp8 is transposed as an fp16 view — partition p
        # of a 256-key pairgroup then holds keys (2p, 2p+1) as adjacent
        # bytes, exactly the [p, 2, q] moving layout DoubleRow contracts.
        # Ragged blocks: block c (pairgroups 4c..4c+3) has q-width
        # (4-c)*128 covering slots m >= c; byte offsets PTOFF.
        WC = [512, 384, 256, 128]  # q-cols per block
        # one tile per block so E block c only waits on the transposes of
        # slots >= c (tile-granular dependency tracking)
        ptc = [
            pt_pool.tile([128, 8 * WC[c]], f8e4, name=f"pt{c}")
            for c in range(4)
        ]

        def pt_view16(m, c):
            # [128, 4 pairgroups, 128] fp16 xbar-dst view of slot m, block c
            w = WC[c]
            span = ptc[c][:].bitcast(f16).rearrange("p (pg w) -> p pg w", pg=4)
            return span[:, :, (m - c) * 128 : (m - c) * 128 + 128]
        # q in fp8 hi/lo (split on-chip from psA), B's DoubleRow stationary
        q8 = [
            qt_pool.tile([128, 4 * 2 * RQ], f8e4, name=f"q8{hl}")
            for hl in range(2)
        ]
        # xtp hi+lo per 1024-key pair of groups: free = (hl, g, i, 1024key)
        xtp_t = [
            xtp_pool.tile([128, 16 * 1024], f8e4, name=f"xtp{j}")
            for j in range(T // 1024)
        ]
        mask_t = mask_pool.tile([128, 4096], f8, name="mask")
        saug_q = mask_pool.tile([128, 2 * RQ], f8e4, name="saug_q")
        saug_k = mask_pool.tile([128, 2 * T], f8e4, name="saug_k")
        wqk_t = [
            wqk_pool.tile([128, KC * 256], f16, name=f"wqk{md2}")
            for md2 in range(KC // 2)
        ]
        xqt_sb = xqt_pool.tile([128, KC * RQ], f16, name="xqt_sb")

        # ---- input DMAs, issued in consumption order ---------------------
        def load_wqk(md2):
            nc.sync.dma_start(
                wqk_t[md2].rearrange("p (kc n) -> p kc n", kc=KC),
                wqk_d[:, md2 * 256 : (md2 + 1) * 256].rearrange(
                    "(kc p) n -> p kc n", p=128
                ),
            )

        def load_xtp(j):
            nc.sync.dma_start(
                xtp_t[j].rearrange("p (s n) -> p s n", s=16),
                xtp8_d[:, j * 1024 : (j + 1) * 1024].rearrange(
                    "(s p) n -> p s n", p=128
                ),
            )

        load_wqk(0)
        nc.sync.dma_start(
            xqt_sb.rearrange("p (kc n) -> p kc n", kc=KC),
            xqt_d.rearrange("(kc p) n -> p kc n", p=128),
        )
        load_wqk(1)
        load_wqk(2)
        load_wqk(3)
        load_xtp(0)
        nc.sync.dma_start(saug_q[0:3, :], saug_q_d)
        nc.sync.dma_start(saug_k[0:3, :], saug_k_d)
        load_xtp(1)
        load_xtp(2)
        nc.sync.dma_start(mask_t, mask_d)
        load_xtp(3)
        nc.sync.dma_start(ident, ident_d)

        # xp prefetch: one contiguous SWDGE-free transfer per 1024-key
        # block. Blocks 3/2 are issued here so they queue on the (FIFO)
        # DMA mutex right behind B's inputs; blocks 1/0 are issued after
        # the slot-2 transposes (below) so the attn transposes of slots
        # 3/2 slip into the mutex queue between them.
        xp_tiles = {3: [None, None], 2: [None, None], 1: [None, None],
                    0: [None, None]}

        def load_xp_part(pool, c, hl):
            xp_t = pool.tile([128, 8 * D], f8e4, name="xp_t", tag="xp")
            nc.sync.dma_start(
                xp_t.rearrange("p (s n) -> p s n", s=8),
                xp8_d[
                    c * 2048 + hl * 1024 : c * 2048 + hl * 1024 + 1024, :
                ].rearrange("(s p) n -> p s n", p=128),
            )
            xp_tiles[c][hl] = xp_t

        # blocks 3/2 + block1-hi get dedicated pre-B tiles (no buffer
        # rotation, so E's first chains never wait on later transfers);
        # block1-lo and block0 load at B's tail from a late pool
        load_xp_part(xpstream, 3, 0)
        load_xp_part(xpstream, 3, 1)
        load_xp_part(xpstream, 2, 0)
        load_xp_part(xpstream, 2, 1)
        load_xp_part(xplate, 1, 0)
        load_xp_part(xplate, 1, 1)

        # PE p-state warmup: the tensor engine downclocks when idle and
        # takes ~3us to re-ramp. Keep it hot with throwaway matmuls into a
        # dedicated PSUM bank while input DMAs land / cross-engine deps
        # resolve. psW is allocated first so its WAR chains stay PE-internal.
        psW = tc.alloc_tile_pool(name="psW", bufs=1, space="PSUM")
        wps = psW.tile([128, 512], f32, name="wps")
        nc.gpsimd.memset(dum[:], 0.0)

        def warm(n):
            for _ in range(n):
                nc.tensor.matmul(
                    wps[:, 0:240], dum[:, 0:128], dum[:], start=True, stop=True
                )

        warm(30)

        # psB is allocated before psA so phase B's first matmuls land in
        # banks disjoint from psA's (no WAR wait on A's last evacuation)
        psB = tc.alloc_tile_pool(name="psB", bufs=4, space="PSUM")

        # ---- Phase A: qT = (xq @ Wqk')^T fp16 (Wqk centered host-side) ---
        # psA chunk mtd holds qT rows 256*(mtd//2)+2p+(mtd%2) via wqk's
        # host column permute; evacuated as q-hi (Act) and q-lo = psA -
        # q-hi (DVE), both e4m3, forming B's DoubleRow pair stationary.
        with tc.tile_pool(name="psA", bufs=2, space="PSUM") as psA:
            for md2 in range(KC // 2):
                for h in range(2):
                    mtd = md2 * 2 + h
                    ps = psA.tile([128, RQ], f32, name="ps_qt")
                    for kc in range(KC):
                        nc.tensor.matmul(
                            ps[:],
                            wqk_t[md2][
                                :, kc * 256 + h * 128 : kc * 256 + h * 128 + 128
                            ],
                            xqt_sb[:, kc * RQ : (kc + 1) * RQ],
                            start=(kc == 0),
                            stop=(kc == KC - 1),
                        )
                    dst = slice(
                        (mtd // 2) * 2 * RQ + (mtd % 2) * RQ,
                        (mtd // 2) * 2 * RQ + (mtd % 2) * RQ + RQ,
                    )
                    nc.scalar.activation(
                        q8[0][:, dst], ps[:], mybir.ActivationFunctionType.Copy
                    )
                    nc.vector.tensor_sub(q8[1][:, dst], ps[:], q8[0][:, dst])
        xqt_pool.release()
        wqk_pool.release()

        # ---- Phase B: per-slot scores + mask + softmax stats + exp/T -----
        # (slot, group) units are independent; they run in xtp-arrival
        # order: slot3's last two groups (the only consumers of the final
        # xtp transfer) are deferred past slot2 so the 8MB xtp stream never
        # stalls the PE. Slot2/3 exps still precede 1/0, matching E's
        # block order.
        p_q_late = {}
        s_tiles = {}
        if True:

            def b_group(m, kg):
                s_t = s_tiles[m]
                ps = psB.tile([128, 512], f32, name="ps_s", tag="psb")
                n = 0
                for hq, hx in ((0, 0), (0, 1), (1, 0)):
                    for g in range(4):
                        stat = q8[hq][
                            :, g * 2 * RQ : (g + 1) * 2 * RQ
                        ].rearrange("p (two n) -> p two n", two=2)[
                            :, :, m * 128 : (m + 1) * 128
                        ]
                        xo = hx * 8192 + g * 2048
                        mov = xtp_t[kg // 2][:, xo : xo + 2048].rearrange(
                            "p (two n) -> p two n", two=2
                        )[:, :, (kg % 2) * 512 : (kg % 2) * 512 + 512]
                        nc.tensor.matmul(
                            ps[:],
                            stat,
                            mov,
                            start=(n == 0),
                            stop=False,
                            perf_mode=DR,
                        )
                        n += 1
                # rank-1 restore: += 0.5*rowsum(x) outer rowsum(x), only
                # the 3 live aug partitions contract (K=3 matmul)
                nc.tensor.matmul(
                    ps[:],
                    saug_q[0:3, :].rearrange("p (two n) -> p two n", two=2)[
                        :, :, m * 128 : (m + 1) * 128
                    ],
                    saug_k[0:3, :].rearrange("p (two n) -> p two n", two=2)[
                        :, :, kg * 512 : (kg + 1) * 512
                    ],
                    start=False,
                    stop=True,
                    perf_mode=DR,
                )
                dst = s_t[:, kg * 512 : (kg + 1) * 512]
                if kg >= BG[m] - 2:
                    mk = kg - (BG[m] - 2)
                    nc.vector.tensor_add(
                        dst,
                        ps[:],
                        mask_t[
                            :, m * 1024 + mk * 512 : m * 1024 + mk * 512 + 512
                        ],
                    )
                else:
                    nc.vector.tensor_copy(dst, ps[:])
                nc.vector.tensor_reduce(
                    mpart[:, MPOFF[m] + kg : MPOFF[m] + kg + 1],
                    dst,
                    axis=mybir.AxisListType.X,
                    op=mybir.AluOpType.max,
                )

            def b_finish(m):
                nc.vector.tensor_reduce(
                    negmax[:, m : m + 1],
                    mpart[:, MPOFF[m] : MPOFF[m] + BG[m]],
                    axis=mybir.AxisListType.X,
                    op=mybir.AluOpType.max,
                    negate=True,
                )
                # exp straight to fp8 attn; transpose the packed-fp16 view
                # through the xbar, one call per 1024-key block
                p_q = p_pool.tile(
                    [128, BKT[m] * 128], f8e4, name="p_q", tag="pq"
                )
                nc.scalar.activation(
                    p_q[:],
                    s_tiles[m][:],
                    mybir.ActivationFunctionType.Exp,
                    bias=negmax[:, m : m + 1],
                    scale=1.0,
                    accum_out=lsum[:, m : m + 1],
                )
                # slots 3/2 transpose through the xbar, issued from the
                # (by now idle) SP queue so ring backpressure from the DMA
                # mutex cannot block the Act sequencer between exps; slots
                # 1/0 finish after the xp-prefetch flood occupies the
                # (serial) DMA engine, so they transpose on the PE below
                if m >= 2:
                    p16 = p_q[:].bitcast(f16)
                    for c in range(NCH[m]):
                        nc.sync.dma_start_transpose(
                            pt_view16(m, c),
                            p16[:, c * 512 : (c + 1) * 512],
                        )
                else:
                    p_q_late[m] = p_q

            # unit order tracks the xtp stream (slot3's tail groups are the
            # only consumers of the last transfers) while keeping slot3's
            # exp FIRST (every E block needs slot3's attn columns) and
            # slots 1/0 last (their transposes run on the PE, post-B)
            s_tiles[3] = s_pool.tile([128, BKT[3] * 128], f32, name="s3", tag="s")
            s_tiles[2] = s_pool.tile([128, BKT[2] * 128], f32, name="s2", tag="s")
            for kg in range(4):
                b_group(3, kg)
            for kg in range(4):
                b_group(2, kg)
            b_group(3, 4)
            b_group(3, 5)
            b_group(2, 4)
            b_group(2, 5)
            b_group(3, 6)
            b_group(3, 7)
            b_finish(3)
            b_finish(2)
            s_tiles[1] = s_pool.tile([128, BKT[1] * 128], f32, name="s1", tag="s")
            for kg in range(4):
                b_group(1, kg)
            b_finish(1)
            s_tiles[0] = s_pool.tile([128, BKT[0] * 128], f32, name="s0", tag="s")
            for kg in range(2):
                b_group(0, kg)
            b_finish(0)
            load_xp_part(xplate, 0, 0)
            load_xp_part(xplate, 0, 1)

        for m in range(4):
            nc.vector.reciprocal(recip[:, m : m + 1], lsum[:, m : m + 1])

        psB.release()
        mask_pool.release()
        xtp_pool.release()
        qt_pool.release()
        s_pool.release()

        # slots 1/0: transpose the packed-fp16 view on the PE (one
        # [128,128] matmul per 256-key pairgroup) into the warmup PSUM
        # bank viewed as fp16, evacuated by the otherwise-idle DVE.
        wps16 = wps[:].bitcast(f16)  # [128, 960]

        def pe_transpose(m):
            p16 = p_q_late[m][:].bitcast(f16)
            npg = BKT[m] // 2
            for pgg in range(npg):
                nc.tensor.matmul(
                    wps16[:, pgg * 128 : (pgg + 1) * 128],
                    p16[:, pgg * 128 : (pgg + 1) * 128],
                    ident,
                    is_transpose=True,
                    start=(pgg == 0),
                    stop=(pgg == npg - 1),
                    skip_group_check=True,
                )
            for c in range(NCH[m]):
                nc.vector.tensor_copy(
                    pt_view16(m, c),
                    wps16[:, c * 512 : (c + 1) * 512].rearrange(
                        "p (pg w) -> p pg w", pg=4
                    ),
                )

        with tc.tile_pool(name="psbr", bufs=1, space="PSUM") as psbr:
            wbr = psbr.tile([128, 512], f32, name="wbr")
            # bridge slot1's exp latency after B's last matmul
            for _ in range(10):
                nc.tensor.matmul(
                    wbr[:, 0:240], dum[:, 0:128], dum[:], start=True, stop=True
                )
            pe_transpose(1)
            # bridge slot0's exp latency with warmup matmuls into a free
            # bank (not the transpose bank: evac reads are still in flight)
            for _ in range(20):
                nc.tensor.matmul(
                    wbr[:, 0:240], dum[:, 0:128], dum[:], start=True, stop=True
                )
            pe_transpose(0)
        psW.release()

        # o1 only exists from E's evacuation on; allocating it here keeps
        # its 8KB out of the B-era SBUF peak
        o1_pool = tc.alloc_tile_pool(name="o1pool", bufs=1)
        o1t = [o1_pool.tile([128, RQ], f16, name=f"o1t{d}") for d in range(KC)]
        wovstream = tc.alloc_tile_pool(name="wovstream", bufs=2)
        wov_t = []
        for nb in range(2):
            wov_blk = wovstream.tile(
                [128, KC * 512], f16, name="wov_blk", tag="wv"
            )
            nc.sync.dma_start(
                wov_blk.rearrange("p (kc n) -> p kc n", kc=KC),
                wov_d[:, nb * 512 : (nb + 1) * 512].rearrange(
                    "(kc p) n -> p kc n", p=128
                ),
            )
            wov_t.append(wov_blk)

        # ---- Phase E: o1T[d] = sum over key pairs, fp8 DoubleRow ---------
        # o1 = attn8 @ (x_hi8 + x_lo8): attn is near-exact in e4m3 (softmax
        # is ~one-hot and exp(0)=1.0 is exact); x carries ~11-bit mantissa
        # via the hi+lo pair. Each matmul contracts a 256-key pairgroup at
        # 0.5 cyc/row (DoubleRow), halving E's tensor time vs fp16.
        with tc.tile_pool(name="psE", bufs=1, space="PSUM") as psE_pool:
            psE = [
                psE_pool.tile([128, RQ], f32, name=f"psE{d}") for d in range(KC)
            ]
            # Blocks largest-key-index first: slots 1-3's attn lands during
            # B (largest-first slot order), so E starts right after B; the
            # slot0-only block 0 runs last, after slot0's post-B exp/xbar
            # (its xp buffer reuses block 3's, whose matmuls finish first).
            for bi, c in enumerate((3, 2, 1, 0)):
                xp_t = xp_tiles[c]
                w = WC[c]
                # the final block runs d-major so each psum bank's chain
                # closes early and its evacuation overlaps E's tail
                if bi == 3:
                    # final block d-major (banks close early for evac
                    # overlap), hi pass before lo per bank (block0's lo
                    # transfer lands mid-E)
                    pdh = [
                        (pg, d, hl)
                        for d in range(KC)
                        for hl in range(2)
                        for pg in range(4)
                    ]
                elif bi == 2:
                    # block 1: all-hi first, its lo transfer lands at B+3us
                    pdh = [
                        (pg, d, hl)
                        for hl in range(2)
                        for pg in range(4)
                        for d in range(KC)
                    ]
                else:
                    # block 3 runs d-descending: banks 7..5 were free since
                    # A, so E's first chains avoid WARs on B's last psum
                    # evacuations (banks 1-4) and the warmup bank (0)
                    dorder = (
                        list(reversed(range(KC))) if bi == 0 else list(range(KC))
                    )
                    pdh = [
                        (pg, d, hl)
                        for pg in range(4)
                        for d in dorder
                        for hl in range(2)
                    ]
                for pg, d, hl in pdh:
                    stat = xp_t[hl][
                        :, pg * 2 * D : (pg + 1) * 2 * D
                    ].rearrange("p (two n) -> p two n", two=2)[
                        :, :, d * 128 : (d + 1) * 128
                    ]
                    mov = ptc[c][
                        :, pg * 2 * w : (pg + 1) * 2 * w
                    ].rearrange("p (q two) -> p two q", two=2)
                    # start_tensor_calc zeroes the WHOLE psum bank, so only
                    # the first matmul into bank d sets it; later slot
                    # regions accumulate onto zeros. All chains end in the
                    # final block (keys 0..1023).
                    nc.tensor.matmul(
                        psE[d][:, c * 128 : 512],
                        stat,
                        mov,
                        start=(bi == 0 and pg == 0 and hl == 0),
                        stop=(c == 0 and pg == 3 and hl == 1),
                        perf_mode=DR,
                        skip_group_check=True,
                    )
            # evacuate: split across DVE and Act so phase F starts sooner
            for d in range(KC):
                if d % 2 == 0:
                    nc.vector.tensor_copy(o1t[d][:], psE[d][:])
                else:
                    nc.scalar.activation(
                        o1t[d][:],
                        psE[d][:],
                        mybir.ActivationFunctionType.Copy,
                    )

        # ---- Phase F: out = (o1 @ Wov) * recip ---------------------------
        with (
            tc.tile_pool(name="psF", bufs=2, space="PSUM") as psF,
            tc.tile_pool(name="outp", bufs=3) as outp,
        ):
            for nb in range(2):
                wov_blk = wov_t[nb]
                for m in range(4):
                    halves = 2 if (nb == 1 and m == 3) else 1
                    w = 512 // halves
                    for h in range(halves):
                        ps = psF.tile([128, w], f32, name="ps_o", tag="pso")
                        for kc in range(KC):
                            nc.tensor.matmul(
                                ps[:],
                                o1t[kc][:, m * 128 : (m + 1) * 128],
                                wov_blk[
                                    :, kc * 512 + h * w : kc * 512 + h * w + w
                                ],
                                start=(kc == 0),
                                stop=(kc == KC - 1),
                            )
                        ob = outp.tile([128, w], f16, name="ob", tag="ob")
                        nc.vector.tensor_scalar_mul(
                            ob[:], ps[:], recip[:, m : m + 1]
                        )
                        nc.sync.dma_start(
                            out_d[
                                m * 128 : (m + 1) * 128,
                                nb * 512 + h * w : nb * 512 + h * w + w,
                            ],
                            ob[:],
                        )

        wovstream.release()
        o1_pool.release()
        p_pool.release()
        xplate.release()
        xpstream.release()
        pt_pool.release()
        consts.release()

    nc.compile()
    return nc


_NC_CACHE = {}


def _get_nc():
    if "nc" not in _NC_CACHE:
        _NC_CACHE["nc"] = _build_nc()
    return _NC_CACHE["nc"]


def _slot_tiles(c):
    return [c, 15 - c, 16 + c, 31 - c]


def _prep_in_maps(x, Wqk, Wov):
    import ml_dtypes

    x = np.ascontiguousarray(np.asarray(x), dtype=np.float32)
    Wqk = np.ascontiguousarray(np.asarray(Wqk), dtype=np.float32)
    Wov = np.ascontiguousarray(np.asarray(Wov), dtype=np.float32)
    wov16 = Wov.astype(np.float16)

    def hilo(a):
        hi = a.astype(ml_dtypes.float8_e4m3)
        lo = (a - hi.astype(np.float32)).astype(ml_dtypes.float8_e4m3)
        return hi, lo

    # contraction-pair row permutation: dram row 256g + 128i + p holds
    # logical row 256g + 2p + i
    def rowperm(n):
        return (
            np.arange(n // 256)[:, None, None] * 256
            + 2 * np.arange(128)[None, None, :]
            + np.arange(2)[None, :, None]
        ).reshape(-1)

    rp_d = rowperm(D)
    # psA output pairing: wqk column mtd*128 + p holds output-d
    # 256*(mtd//2) + 2p + (mtd%2)
    colp = (
        np.arange(KC)[:, None] // 2 * 256
        + 2 * np.arange(128)[None, :]
        + np.arange(KC)[:, None] % 2
    ).reshape(-1)

    # centered Wqk (halves fp8 reconstruction error downstream), fp16,
    # columns permuted for the q8 pair layout
    wqkc = np.ascontiguousarray((Wqk - 0.5)[:, colp].astype(np.float16))
    xT = np.ascontiguousarray(x.T)  # [D, T] fp32
    xT16 = xT.astype(np.float16)
    xtph, xtpl = hilo(xT[rp_d])
    xtp8 = np.ascontiguousarray(np.concatenate([xtph, xtpl], axis=0))

    # rank-1 restore: 0.5*sx(i)*sx(j) via triple e4m3 split of sx
    sx = x.sum(1).astype(np.float32)
    s0 = sx.astype(ml_dtypes.float8_e4m3)
    r1 = sx - s0.astype(np.float32)
    s1 = r1.astype(ml_dtypes.float8_e4m3)
    s2 = (r1 - s1.astype(np.float32)).astype(ml_dtypes.float8_e4m3)
    sq = [q.astype(np.float32) for q in (s0, s1, s2)]
    APAIR = [(0, 0), (0, 1), (1, 0), (1, 1), (0, 2), (2, 0)]  # (a-part, b-part)
    saug_k = np.zeros((3, 2, T), dtype=ml_dtypes.float8_e4m3)
    for e, (ai, bi) in enumerate(APAIR):
        saug_k[e // 2, e % 2, :] = sq[bi].astype(ml_dtypes.float8_e4m3)

    # x rows as fp8 hi + lo (hi+lo carries ~11-bit mantissa), rows permuted
    # so dram row 256g + 128i + p holds logical row 256g + 2p + i: after the
    # pairgroup DMA, partition p carries rows (2p, 2p+1) of its pairgroup as
    # DoubleRow contraction pairs.
    xh8 = x.astype(ml_dtypes.float8_e4m3)
    xl8 = (x - xh8.astype(np.float32)).astype(ml_dtypes.float8_e4m3)
    # logical row for dram slot (c, pg, i, p): 1024c + 256pg + 2p + i
    idx = (
        np.arange(T // 256)[:, None, None] * 256
        + 2 * np.arange(128)[None, None, :]
        + np.arange(2)[None, :, None]
    ).reshape(-1, 1024)  # [block c, row-in-block]
    xp8 = np.empty((2 * T, D), dtype=ml_dtypes.float8_e4m3)
    for c in range(4):
        xp8[c * 2048 : c * 2048 + 1024] = xh8[idx[c]]
        xp8[c * 2048 + 1024 : c * 2048 + 2048] = xl8[idx[c]]

    in_maps = []
    for c in range(NCORES):
        tiles = _slot_tiles(c)
        rows = np.concatenate(
            [np.arange(t * 128, (t + 1) * 128) for t in tiles]
        )
        xqt = np.ascontiguousarray(xT16[:, rows])
        saug_q = np.zeros((3, 2, RQ), dtype=ml_dtypes.float8_e4m3)
        for e, (ai, bi) in enumerate(APAIR):
            saug_q[e // 2, e % 2, :] = (0.5 * sq[ai][rows]).astype(
                ml_dtypes.float8_e4m3
            )

        mask = np.full((128, 4096), NEG16, dtype=ml_dtypes.float8_e5m2)
        p = np.arange(128)[:, None]
        for m, t in enumerate(tiles):
            g = t * 128 + p  # global row index per partition
            # last 1024 keys of slot m's budget: [1024m, 1024(m+1))
            y = 1024 * m + np.arange(1024)[None, :]
            mask[:, m * 1024 : (m + 1) * 1024] = np.where(
                y <= g, 0.0, NEG16
            ).astype(ml_dtypes.float8_e5m2)
        in_maps.append(
            {
                "xqt": xqt,
                "xtp8": xtp8,
                "xp8": xp8,
                "wqk": wqkc,
                "wov": wov16,
                "mask": mask,
                "ident": np.eye(128, dtype=np.float16),
                "saug_q": np.ascontiguousarray(saug_q.reshape(3, 2 * RQ)),
                "saug_k": np.ascontiguousarray(saug_k.reshape(3, 2 * T)),
            }
        )
    return in_maps


def run(x, Wqk, Wov, **spmd_kwargs):
    """Full pipeline; returns (output [T, D] fp32, BassKernelResults)."""
    import time

    nc = _get_nc()
    in_maps = _prep_in_maps(x, Wqk, Wov)
    try:
        res = run_bass_kernel_spmd(
            nc, in_maps, core_ids=list(range(NCORES)), **spmd_kwargs
        )
    except Exception:
        # a prior crashed execution can leave a core transiently
        # unrecoverable; the runtime resets it — retry once
        time.sleep(10)
        res = run_bass_kernel_spmd(
            nc, in_maps, core_ids=list(range(NCORES)), **spmd_kwargs
        )
    out = np.empty((T, D), dtype=np.float32)
    for c in range(NCORES):
        co = res.results[c]["out"]
        for m, t in enumerate(_slot_tiles(c)):
            out[t * 128 : (t + 1) * 128] = co[m * 128 : (m + 1) * 128]
    return np.ascontiguousarray(out), res


def kernel(x, Wqk, Wov):
    out, _ = run(x, Wqk, Wov)
    return out



# revision 125
# speedup vs baseline: 1.0406x; 1.0406x over previous
"""Causal attention kernel for Trainium2, 8 NeuronCores, sequence-parallel.

Reference computation (T=4096, D=1024, fp32):
    q = x @ Wqk; logits = q @ x.T (causal masked); attn = softmax(logits)
    out = (attn @ x) @ Wov

Causal load balancing under one SPMD program: the 32 query row-tiles of 128
are assigned to cores as {c, 15-c, 16+c, 31-c} and host-permuted into 4
local "slots" ordered by visibility class. Slot m processes a fixed key
budget of 8*(m+1) key-tiles (keys in natural order, prefix [0, 1024*(m+1))),
which covers every core's visible range in that class. Causality inside the
budget is enforced by a host-provided additive mask (0 / -60000) that also
carries the diagonal triangle, so the program is core-independent while
skipping 37.5% of the score/AV matmul work.

Matmul precision: fp16 inputs (x, Wqk, Wov, attn) with fp32 PSUM
accumulation; q and o1 kept in fp16 on-chip. Softmax row max subtracted in
fp32; attn stored fp16 for the DMA-xbar transposes and AV.

Scheduling notes: input DMAs are issued in consumption order (xqt/wqk for
phase A first, then keys/masks); phase B runs slots largest-first so the
exp/transpose pipeline drains during B and phase E can start right after;
tiles are split per dependency unit (per-kg keys, per-chunk attn-transpose,
per-d o1) to keep cross-engine waits granular.
"""

import sys

sys.path.insert(0, "/opt/trn_rl_repo")

import numpy as np

import concourse.tile as tile
from concourse import bacc, mybir
from concourse.bass_utils import run_bass_kernel_spmd

T = 4096
D = 1024
NCORES = 8
RQ = T // NCORES  # 512 query rows per core
KC = D // 128  # 8 contraction chunks
NEG16 = -57344.0  # exactly representable in fp8e5m2

BKT = [8, 16, 24, 32]  # key tiles (128) processed per slot
BG = [b // 4 for b in BKT]  # 512-wide key groups per slot
OFFK = [0, 1024, 3072, 6144]  # slot column offsets in ragged score layout
STOT = 10240  # total score/mask columns
MPOFF = [0, 2, 6, 12]  # mpart offsets (prefix of BG)
NCH = [b // 8 for b in BKT]  # 1024-wide exp chunks per slot: 1,2,3,4
LQOFF = [0, 1, 3, 6]  # lq offsets (prefix of NCH)

f32 = mybir.dt.float32
f16 = mybir.dt.float16
f8 = mybir.dt.float8e5
f8e4 = mybir.dt.float8e4
DR = mybir.MatmulPerfMode.DoubleRow


def _build_nc():
    nc = bacc.Bacc(
        "TRN2", target_bir_lowering=False, debug=False, num_devices=NCORES
    )

    # B's moving operand in fp8 hi/lo with contraction-paired rows: dram
    # row 256g + 128i + p holds contraction index 256g + 2p + i (i is the
    # DoubleRow pair element); hi block then lo block per 1024-key column
    # group. Phase A stays fp16 (on the CENTERED Wqk, see below).
    xqt_d = nc.dram_tensor("xqt", [D, RQ], f16, kind="ExternalInput").ap()
    xtp8_d = nc.dram_tensor("xtp8", [2 * D, T], f8e4, kind="ExternalInput").ap()
    # rank-1 restore operands: Wqk is centered host-side (W' = Wqk - 0.5)
    # to halve fp8 reconstruction error; the dropped 0.5*rowsum(x) outer
    # product is restored by one extra DoubleRow matmul per score group
    # using a triple e4m3 split of rowsum(x) (6 cross products, 3 of the
    # 128 aug partitions used, rest zero).
    saug_q_d = nc.dram_tensor("saug_q", [3, 2 * RQ], f8e4, kind="ExternalInput").ap()
    saug_k_d = nc.dram_tensor("saug_k", [3, 2 * T], f8e4, kind="ExternalInput").ap()
    # x rows in fp8 hi/lo, host-permuted block-major: dram row
    # 2048c + 1024hl + 256pg + 128i + p holds hi/lo of logical row
    # 1024c + 256pg + 2p + i, so each block is one contiguous transfer and
    # partition p carries rows (2p, 2p+1) as DoubleRow contraction pairs
    xp8_d = nc.dram_tensor("xp8", [2 * T, D], f8e4, kind="ExternalInput").ap()
    # wqk (centered, fp16) columns are host-permuted so psA chunk mtd
    # yields qT rows d = 256*(mtd//2) + 2p + (mtd%2): partition p of the
    # evacuated q-hi/lo then carries the DoubleRow pair (2p, 2p+1) for B
    wqk_d = nc.dram_tensor("wqk", [D, D], f16, kind="ExternalInput").ap()
    wov_d = nc.dram_tensor("wov", [D, D], f16, kind="ExternalInput").ap()
    ident_d = nc.dram_tensor("ident", [128, 128], f16, kind="ExternalInput").ap()
    # causal mask, only the last 1024 keys of each slot's budget: for class
    # m (tiles 8m..8m+7) keys below 1024m are visible to every member core,
    # so only the final window carries the triangle / -inf region
    mask_d = nc.dram_tensor("mask", [128, 4096], f8, kind="ExternalInput").ap()
    out_d = nc.dram_tensor("out", [RQ, D], f16, kind="ExternalOutput").ap()

    with tile.TileContext(nc) as tc:
        # stack allocator: long-lived pools first
        consts = tc.alloc_tile_pool(name="consts", bufs=1)
        pt_pool = tc.alloc_tile_pool(name="ptpool", bufs=1)
        xpstream = tc.alloc_tile_pool(name="xpstream", bufs=4)
        xplate = tc.alloc_tile_pool(name="xplate", bufs=2)
        p_pool = tc.alloc_tile_pool(name="ppool", bufs=2)
        s_pool = tc.alloc_tile_pool(name="spool", bufs=2)
        qt_pool = tc.alloc_tile_pool(name="qt", bufs=1)
        xtp_pool = tc.alloc_tile_pool(name="xtpp", bufs=1)
        mask_pool = tc.alloc_tile_pool(name="maskp", bufs=1)
        wqk_pool = tc.alloc_tile_pool(name="wqkp", bufs=1)
        xqt_pool = tc.alloc_tile_pool(name="xqtp", bufs=1)

        # stats scratch: negmax 0:4, lsum 4:8, recip 8:12, mpart 12:32
        smalls = consts.tile([128, 32], f32, name="smalls")
        dum = consts.tile([128, 240], f16, name="dum")
        ident = consts.tile([128, 128], f16, name="ident")
        negmax = smalls[:, 0:4]
        lsum = smalls[:, 4:8]
        recip = smalls[:, 8:12]
        mpart = smalls[:, 12:32]

        # transposed attn, fp8, packed in key-PAIRS: the xbar moves 2-byte
        # granules, so attn fp8 is transposed as an fp16 view — partition p
        # of a 256-key pairgroup then holds keys (2p, 2p+1) as adjacent
        # bytes, exactly the [p, 2, q] moving layout DoubleRow contracts.
        # Ragged blocks: block c (pairgroups 4c..4c+3) has q-width
        # (4-c)*128 covering slots m >= c; byte offsets PTOFF.
        WC = [512, 384, 256, 128]  # q-cols per block
        # one tile per block so E block c only waits on the transposes of
        # slots >= c (tile-granular dependency tracking)
        ptc = [
            pt_pool.tile([128, 8 * WC[c]], f8e4, name=f"pt{c}")
            for c in range(4)
        ]

        def pt_view16(m, c):
            # [128, 4 pairgroups, 128] fp16 xbar-dst view of slot m, block c
            w = WC[c]
            span = ptc[c][:].bitcast(f16).rearrange("p (pg w) -> p pg w", pg=4)
            return span[:, :, (m - c) * 128 : (m - c) * 128 + 128]
        # q in fp8 hi/lo (split on-chip from psA), B's DoubleRow stationary
        q8 = [
            qt_pool.tile([128, 4 * 2 * RQ], f8e4, name=f"q8{hl}")
            for hl in range(2)
        ]
        # xtp hi+lo per 1024-key pair of groups: free = (hl, g, i, 1024key)
        xtp_t = [
            xtp_pool.tile([128, 16 * 1024], f8e4, name=f"xtp{j}")
            for j in range(T // 1024)
        ]
        mask_t = mask_pool.tile([128, 4096], f8, name="mask")
        saug_q = mask_pool.tile([128, 2 * RQ], f8e4, name="saug_q")
        saug_k = mask_pool.tile([128, 2 * T], f8e4, name="saug_k")
        wqk_t = [
            wqk_pool.tile([128, KC * 256], f16, name=f"wqk{md2}")
            for md2 in range(KC // 2)
        ]
        xqt_sb = xqt_pool.tile([128, KC * RQ], f16, name="xqt_sb")

        # ---- input DMAs, issued in consumption order ---------------------
        def load_wqk(md2):
            nc.sync.dma_start(
                wqk_t[md2].rearrange("p (kc n) -> p kc n", kc=KC),
                wqk_d[:, md2 * 256 : (md2 + 1) * 256].rearrange(
                    "(kc p) n -> p kc n", p=128
                ),
            )

        def load_xtp(j):
            nc.sync.dma_start(
                xtp_t[j].rearrange("p (s n) -> p s n", s=16),
                xtp8_d[:, j * 1024 : (j + 1) * 1024].rearrange(
                    "(s p) n -> p s n", p=128
                ),
            )

        load_wqk(0)
        nc.sync.dma_start(
            xqt_sb.rearrange("p (kc n) -> p kc n", kc=KC),
            xqt_d.rearrange("(kc p) n -> p kc n", p=128),
        )
        load_wqk(1)
        load_wqk(2)
        load_wqk(3)
        load_xtp(0)
        nc.sync.dma_start(saug_q[0:3, :], saug_q_d)
        nc.sync.dma_start(saug_k[0:3, :], saug_k_d)
        load_xtp(1)
        load_xtp(2)
        nc.sync.dma_start(mask_t, mask_d)
        load_xtp(3)
        nc.sync.dma_start(ident, ident_d)

        # xp prefetch: one contiguous SWDGE-free transfer per 1024-key
        # block. Blocks 3/2 are issued here so they queue on the (FIFO)
        # DMA mutex right behind B's inputs; blocks 1/0 are issued after
        # the slot-2 transposes (below) so the attn transposes of slots
        # 3/2 slip into the mutex queue between them.
        xp_tiles = {3: [None, None], 2: [None, None], 1: [None, None],
                    0: [None, None]}

        def load_xp_part(pool, c, hl):
            xp_t = pool.tile([128, 8 * D], f8e4, name="xp_t", tag="xp")
            nc.sync.dma_start(
                xp_t.rearrange("p (s n) -> p s n", s=8),
                xp8_d[
                    c * 2048 + hl * 1024 : c * 2048 + hl * 1024 + 1024, :
                ].rearrange("(s p) n -> p s n", p=128),
            )
            xp_tiles[c][hl] = xp_t

        # blocks 3/2 + block1-hi get dedicated pre-B tiles (no buffer
        # rotation, so E's first chains never wait on later transfers);
        # block1-lo and block0 load at B's tail from a late pool
        load_xp_part(xpstream, 3, 0)
        load_xp_part(xpstream, 3, 1)
        load_xp_part(xpstream, 2, 0)
        load_xp_part(xpstream, 2, 1)
        load_xp_part(xplate, 1, 0)

        # PE p-state warmup: the tensor engine downclocks when idle and
        # takes ~3us to re-ramp. Keep it hot with throwaway matmuls into a
        # dedicated PSUM bank while input DMAs land / cross-engine deps
        # resolve. psW is allocated first so its WAR chains stay PE-internal.
        psW = tc.alloc_tile_pool(name="psW", bufs=1, space="PSUM")
        wps = psW.tile([128, 512], f32, name="wps")
        nc.gpsimd.memset(dum[:], 0.0)

        def warm(n):
            for _ in range(n):
                nc.tensor.matmul(
                    wps[:, 0:240], dum[:, 0:128], dum[:], start=True, stop=True
                )

        warm(30)

        # psB is allocated before psA so phase B's first matmuls land in
        # banks disjoint from psA's (no WAR wait on A's last evacuation)
        psB = tc.alloc_tile_pool(name="psB", bufs=4, space="PSUM")

        # ---- Phase A: qT = (xq @ Wqk')^T fp16 (Wqk centered host-side) ---
        # psA chunk mtd holds qT rows 256*(mtd//2)+2p+(mtd%2) via wqk's
        # host column permute; evacuated as q-hi (Act) and q-lo = psA -
        # q-hi (DVE), both e4m3, forming B's DoubleRow pair stationary.
        with tc.tile_pool(name="psA", bufs=2, space="PSUM") as psA:
            for md2 in range(KC // 2):
                for h in range(2):
                    mtd = md2 * 2 + h
                    ps = psA.tile([128, RQ], f32, name="ps_qt")
                    for kc in range(KC):
                        nc.tensor.matmul(
                            ps[:],
                            wqk_t[md2][
                                :, kc * 256 + h * 128 : kc * 256 + h * 128 + 128
                            ],
                            xqt_sb[:, kc * RQ : (kc + 1) * RQ],
                            start=(kc == 0),
                            stop=(kc == KC - 1),
                        )
                    dst = slice(
                        (mtd // 2) * 2 * RQ + (mtd % 2) * RQ,
                        (mtd // 2) * 2 * RQ + (mtd % 2) * RQ + RQ,
                    )
                    nc.scalar.activation(
                        q8[0][:, dst], ps[:], mybir.ActivationFunctionType.Copy
                    )
                    nc.vector.tensor_sub(q8[1][:, dst], ps[:], q8[0][:, dst])
        xqt_pool.release()
        wqk_pool.release()

        # ---- Phase B: per-slot scores + mask + softmax stats + exp/T -----
        # (slot, group) units are independent; they run in xtp-arrival
        # order: slot3's last two groups (the only consumers of the final
        # xtp transfer) are deferred past slot2 so the 8MB xtp stream never
        # stalls the PE. Slot2/3 exps still precede 1/0, matching E's
        # block order.
        p_q_late = {}
        s_tiles = {}
        if True:

            def b_group(m, kg):
                s_t = s_tiles[m]
                ps = psB.tile([128, 512], f32, name="ps_s", tag="psb")
                n = 0
                for hq, hx in ((0, 0), (0, 1), (1, 0)):
                    for g in range(4):
                        stat = q8[hq][
                            :, g * 2 * RQ : (g + 1) * 2 * RQ
                        ].rearrange("p (two n) -> p two n", two=2)[
                            :, :, m * 128 : (m + 1) * 128
                        ]
                        xo = hx * 8192 + g * 2048
                        mov = xtp_t[kg // 2][:, xo : xo + 2048].rearrange(
                            "p (two n) -> p two n", two=2
                        )[:, :, (kg % 2) * 512 : (kg % 2) * 512 + 512]
                        nc.tensor.matmul(
                            ps[:],
                            stat,
                            mov,
                            start=(n == 0),
                            stop=False,
                            perf_mode=DR,
                        )
                        n += 1
                # rank-1 restore: += 0.5*rowsum(x) outer rowsum(x), only
                # the 3 live aug partitions contract (K=3 matmul)
                nc.tensor.matmul(
                    ps[:],
                    saug_q[0:3, :].rearrange("p (two n) -> p two n", two=2)[
                        :, :, m * 128 : (m + 1) * 128
                    ],
                    saug_k[0:3, :].rearrange("p (two n) -> p two n", two=2)[
                        :, :, kg * 512 : (kg + 1) * 512
                    ],
                    start=False,
                    stop=True,
                    perf_mode=DR,
                )
                dst = s_t[:, kg * 512 : (kg + 1) * 512]
                if kg >= BG[m] - 2:
                    mk = kg - (BG[m] - 2)
                    nc.vector.tensor_add(
                        dst,
                        ps[:],
                        mask_t[
                            :, m * 1024 + mk * 512 : m * 1024 + mk * 512 + 512
                        ],
                    )
                else:
                    nc.vector.tensor_copy(dst, ps[:])
                nc.vector.tensor_reduce(
                    mpart[:, MPOFF[m] + kg : MPOFF[m] + kg + 1],
                    dst,
                    axis=mybir.AxisListType.X,
                    op=mybir.AluOpType.max,
                )

            def b_finish(m):
                nc.vector.tensor_reduce(
                    negmax[:, m : m + 1],
                    mpart[:, MPOFF[m] : MPOFF[m] + BG[m]],
                    axis=mybir.AxisListType.X,
                    op=mybir.AluOpType.max,
                    negate=True,
                )
                # exp straight to fp8 attn; transpose the packed-fp16 view
                # through the xbar, one call per 1024-key block
                p_q = p_pool.tile(
                    [128, BKT[m] * 128], f8e4, name="p_q", tag="pq"
                )
                nc.scalar.activation(
                    p_q[:],
                    s_tiles[m][:],
                    mybir.ActivationFunctionType.Exp,
                    bias=negmax[:, m : m + 1],
                    scale=1.0,
                    accum_out=lsum[:, m : m + 1],
                )
                # slots 3/2 transpose through the xbar, issued from the
                # (by now idle) SP queue so ring backpressure from the DMA
                # mutex cannot block the Act sequencer between exps; slots
                # 1/0 finish after the xp-prefetch flood occupies the
                # (serial) DMA engine, so they transpose on the PE below
                if m >= 2:
                    p16 = p_q[:].bitcast(f16)
                    for c in range(NCH[m]):
                        nc.sync.dma_start_transpose(
                            pt_view16(m, c),
                            p16[:, c * 512 : (c + 1) * 512],
                        )
                else:
                    p_q_late[m] = p_q

            # unit order tracks the xtp stream (slot3's tail groups are the
            # only consumers of the last transfers) while keeping slot3's
            # exp FIRST (every E block needs slot3's attn columns) and
            # slots 1/0 last (their transposes run on the PE, post-B)
            s_tiles[3] = s_pool.tile([128, BKT[3] * 128], f32, name="s3", tag="s")
            s_tiles[2] = s_pool.tile([128, BKT[2] * 128], f32, name="s2", tag="s")
            for kg in range(4):
                b_group(3, kg)
            for kg in range(4):
                b_group(2, kg)
            b_group(3, 4)
            b_group(3, 5)
            b_group(2, 4)
            b_group(2, 5)
            b_group(3, 6)
            b_group(3, 7)
            b_finish(3)
            b_finish(2)
            s_tiles[1] = s_pool.tile([128, BKT[1] * 128], f32, name="s1", tag="s")
            for kg in range(4):
                b_group(1, kg)
            b_finish(1)
            # block1-lo issues right after slot1's transposes so its
            # transfer overlaps B's tail; E reads block1 hi-pass first
            load_xp_part(xplate, 1, 1)
            s_tiles[0] = s_pool.tile([128, BKT[0] * 128], f32, name="s0", tag="s")
            for kg in range(2):
                b_group(0, kg)
            b_finish(0)
            load_xp_part(xplate, 0, 0)
            load_xp_part(xplate, 0, 1)

        for m in range(4):
            nc.vector.reciprocal(recip[:, m : m + 1], lsum[:, m : m + 1])

        psB.release()
        mask_pool.release()
        xtp_pool.release()
        qt_pool.release()
        s_pool.release()

        # slots 1/0: transpose the packed-fp16 view on the PE (one
        # [128,128] matmul per 256-key pairgroup) into the warmup PSUM
        # bank viewed as fp16, evacuated by the otherwise-idle DVE.
        wps16 = wps[:].bitcast(f16)  # [128, 960]

        def pe_transpose(m):
            p16 = p_q_late[m][:].bitcast(f16)
            npg = BKT[m] // 2
            for pgg in range(npg):
                nc.tensor.matmul(
                    wps16[:, pgg * 128 : (pgg + 1) * 128],
                    p16[:, pgg * 128 : (pgg + 1) * 128],
                    ident,
                    is_transpose=True,
                    start=(pgg == 0),
                    stop=(pgg == npg - 1),
                    skip_group_check=True,
                )
            for c in range(NCH[m]):
                nc.vector.tensor_copy(
                    pt_view16(m, c),
                    wps16[:, c * 512 : (c + 1) * 512].rearrange(
                        "p (pg w) -> p pg w", pg=4
                    ),
                )

        with tc.tile_pool(name="psbr", bufs=1, space="PSUM") as psbr:
            wbr = psbr.tile([128, 512], f32, name="wbr")
            # bridge slot1's exp latency after B's last matmul
            for _ in range(10):
                nc.tensor.matmul(
                    wbr[:, 0:240], dum[:, 0:128], dum[:], start=True, stop=True
                )
            pe_transpose(1)
            # bridge slot0's exp latency with warmup matmuls into a free
            # bank (not the transpose bank: evac reads are still in flight)
            for _ in range(20):
                nc.tensor.matmul(
                    wbr[:, 0:240], dum[:, 0:128], dum[:], start=True, stop=True
                )
            pe_transpose(0)
        psW.release()

        # o1 only exists from E's evacuation on; allocating it here keeps
        # its 8KB out of the B-era SBUF peak
        o1_pool = tc.alloc_tile_pool(name="o1pool", bufs=1)
        o1t = [o1_pool.tile([128, RQ], f16, name=f"o1t{d}") for d in range(KC)]
        wovstream = tc.alloc_tile_pool(name="wovstream", bufs=2)
        wov_t = []
        for nb in range(2):
            wov_blk = wovstream.tile(
                [128, KC * 512], f16, name="wov_blk", tag="wv"
            )
            nc.sync.dma_start(
                wov_blk.rearrange("p (kc n) -> p kc n", kc=KC),
                wov_d[:, nb * 512 : (nb + 1) * 512].rearrange(
                    "(kc p) n -> p kc n", p=128
                ),
            )
            wov_t.append(wov_blk)

        # ---- Phase E: o1T[d] = sum over key pairs, fp8 DoubleRow ---------
        # o1 = attn8 @ (x_hi8 + x_lo8): attn is near-exact in e4m3 (softmax
        # is ~one-hot and exp(0)=1.0 is exact); x carries ~11-bit mantissa
        # via the hi+lo pair. Each matmul contracts a 256-key pairgroup at
        # 0.5 cyc/row (DoubleRow), halving E's tensor time vs fp16.
        with tc.tile_pool(name="psE", bufs=1, space="PSUM") as psE_pool:
            psE = [
                psE_pool.tile([128, RQ], f32, name=f"psE{d}") for d in range(KC)
            ]
            # Blocks largest-key-index first: slots 1-3's attn lands during
            # B (largest-first slot order), so E starts right after B; the
            # slot0-only block 0 runs last, after slot0's post-B exp/xbar
            # (its xp buffer reuses block 3's, whose matmuls finish first).
            for bi, c in enumerate((3, 2, 1, 0)):
                xp_t = xp_tiles[c]
                w = WC[c]
                # the final block runs d-major so each psum bank's chain
                # closes early and its evacuation overlaps E's tail
                if bi == 3:
                    # final block d-major (banks close early for evac
                    # overlap), hi pass before lo per bank (block0's lo
                    # transfer lands mid-E)
                    pdh = [
                        (pg, d, hl)
                        for d in range(KC)
                        for hl in range(2)
                        for pg in range(4)
                    ]
                elif bi == 2:
                    # block 1: all-hi first, its lo transfer lands at B+3us
                    pdh = [
                        (pg, d, hl)
                        for hl in range(2)
                        for pg in range(4)
                        for d in range(KC)
                    ]
                else:
                    # block 3 runs d-descending: banks 7..5 were free since
                    # A, so E's first chains avoid WARs on B's last psum
                    # evacuations (banks 1-4) and the warmup bank (0)
                    dorder = (
                        list(reversed(range(KC))) if bi == 0 else list(range(KC))
                    )
                    pdh = [
                        (pg, d, hl)
                        for pg in range(4)
                        for d in dorder
                        for hl in range(2)
                    ]
                for pg, d, hl in pdh:
                    stat = xp_t[hl][
                        :, pg * 2 * D : (pg + 1) * 2 * D
                    ].rearrange("p (two n) -> p two n", two=2)[
                        :, :, d * 128 : (d + 1) * 128
                    ]
                    mov = ptc[c][
                        :, pg * 2 * w : (pg + 1) * 2 * w
                    ].rearrange("p (q two) -> p two q", two=2)
                    # start_tensor_calc zeroes the WHOLE psum bank, so only
                    # the first matmul into bank d sets it; later slot
                    # regions accumulate onto zeros. All chains end in the
                    # final block (keys 0..1023).
                    nc.tensor.matmul(
                        psE[d][:, c * 128 : 512],
                        stat,
                        mov,
                        start=(bi == 0 and pg == 0 and hl == 0),
                        stop=(c == 0 and pg == 3 and hl == 1),
                        perf_mode=DR,
                        skip_group_check=True,
                    )
            # evacuate: split across DVE and Act so phase F starts sooner
            for d in range(KC):
                if d % 2 == 0:
                    nc.vector.tensor_copy(o1t[d][:], psE[d][:])
                else:
                    nc.scalar.activation(
                        o1t[d][:],
                        psE[d][:],
                        mybir.ActivationFunctionType.Copy,
                    )

        # ---- Phase F: out = (o1 @ Wov) * recip ---------------------------
        with (
            tc.tile_pool(name="psF", bufs=2, space="PSUM") as psF,
            tc.tile_pool(name="outp", bufs=3) as outp,
        ):
            for nb in range(2):
                wov_blk = wov_t[nb]
                for m in range(4):
                    halves = 2 if (nb == 1 and m == 3) else 1
                    w = 512 // halves
                    for h in range(halves):
                        ps = psF.tile([128, w], f32, name="ps_o", tag="pso")
                        for kc in range(KC):
                            nc.tensor.matmul(
                                ps[:],
                                o1t[kc][:, m * 128 : (m + 1) * 128],
                                wov_blk[
                                    :, kc * 512 + h * w : kc * 512 + h * w + w
                                ],
                                start=(kc == 0),
                                stop=(kc == KC - 1),
                            )
                        ob = outp.tile([128, w], f16, name="ob", tag="ob")
                        nc.vector.tensor_scalar_mul(
                            ob[:], ps[:], recip[:, m : m + 1]
                        )
                        nc.sync.dma_start(
                            out_d[
                                m * 128 : (m + 1) * 128,
                                nb * 512 + h * w : nb * 512 + h * w + w,
                            ],
                            ob[:],
                        )

        wovstream.release()
        o1_pool.release()
        p_pool.release()
        xplate.release()
        xpstream.release()
        pt_pool.release()
        consts.release()

    nc.compile()
    return nc


_NC_CACHE = {}


def _get_nc():
    if "nc" not in _NC_CACHE:
        _NC_CACHE["nc"] = _build_nc()
    return _NC_CACHE["nc"]


def _slot_tiles(c):
    return [c, 15 - c, 16 + c, 31 - c]


def _prep_in_maps(x, Wqk, Wov):
    import ml_dtypes

    x = np.ascontiguousarray(np.asarray(x), dtype=np.float32)
    Wqk = np.ascontiguousarray(np.asarray(Wqk), dtype=np.float32)
    Wov = np.ascontiguousarray(np.asarray(Wov), dtype=np.float32)
    wov16 = Wov.astype(np.float16)

    def hilo(a):
        hi = a.astype(ml_dtypes.float8_e4m3)
        lo = (a - hi.astype(np.float32)).astype(ml_dtypes.float8_e4m3)
        return hi, lo

    # contraction-pair row permutation: dram row 256g + 128i + p holds
    # logical row 256g + 2p + i
    def rowperm(n):
        return (
            np.arange(n // 256)[:, None, None] * 256
            + 2 * np.arange(128)[None, None, :]
            + np.arange(2)[None, :, None]
        ).reshape(-1)

    rp_d = rowperm(D)
    # psA output pairing: wqk column mtd*128 + p holds output-d
    # 256*(mtd//2) + 2p + (mtd%2)
    colp = (
        np.arange(KC)[:, None] // 2 * 256
        + 2 * np.arange(128)[None, :]
        + np.arange(KC)[:, None] % 2
    ).reshape(-1)

    # centered Wqk (halves fp8 reconstruction error downstream), fp16,
    # columns permuted for the q8 pair layout
    wqkc = np.ascontiguousarray((Wqk - 0.5)[:, colp].astype(np.float16))
    xT = np.ascontiguousarray(x.T)  # [D, T] fp32
    xT16 = xT.astype(np.float16)
    xtph, xtpl = hilo(xT[rp_d])
    xtp8 = np.ascontiguousarray(np.concatenate([xtph, xtpl], axis=0))

    # rank-1 restore: 0.5*sx(i)*sx(j) via triple e4m3 split of sx
    sx = x.sum(1).astype(np.float32)
    s0 = sx.astype(ml_dtypes.float8_e4m3)
    r1 = sx - s0.astype(np.float32)
    s1 = r1.astype(ml_dtypes.float8_e4m3)
    s2 = (r1 - s1.astype(np.float32)).astype(ml_dtypes.float8_e4m3)
    sq = [q.astype(np.float32) for q in (s0, s1, s2)]
    APAIR = [(0, 0), (0, 1), (1, 0), (1, 1), (0, 2), (2, 0)]  # (a-part, b-part)
    saug_k = np.zeros((3, 2, T), dtype=ml_dtypes.float8_e4m3)
    for e, (ai, bi) in enumerate(APAIR):
        saug_k[e // 2, e % 2, :] = sq[bi].astype(ml_dtypes.float8_e4m3)

    # x rows as fp8 hi + lo (hi+lo carries ~11-bit mantissa), rows permuted
    # so dram row 256g + 128i + p holds logical row 256g + 2p + i: after the
    # pairgroup DMA, partition p carries rows (2p, 2p+1) of its pairgroup as
    # DoubleRow contraction pairs.
    xh8 = x.astype(ml_dtypes.float8_e4m3)
    xl8 = (x - xh8.astype(np.float32)).astype(ml_dtypes.float8_e4m3)
    # logical row for dram slot (c, pg, i, p): 1024c + 256pg + 2p + i
    idx = (
        np.arange(T // 256)[:, None, None] * 256
        + 2 * np.arange(128)[None, None, :]
        + np.arange(2)[None, :, None]
    ).reshape(-1, 1024)  # [block c, row-in-block]
    xp8 = np.empty((2 * T, D), dtype=ml_dtypes.float8_e4m3)
    for c in range(4):
        xp8[c * 2048 : c * 2048 + 1024] = xh8[idx[c]]
        xp8[c * 2048 + 1024 : c * 2048 + 2048] = xl8[idx[c]]

    in_maps = []
    for c in range(NCORES):
        tiles = _slot_tiles(c)
        rows = np.concatenate(
            [np.arange(t * 128, (t + 1) * 128) for t in tiles]
        )
        xqt = np.ascontiguousarray(xT16[:, rows])
        saug_q = np.zeros((3, 2, RQ), dtype=ml_dtypes.float8_e4m3)
        for e, (ai, bi) in enumerate(APAIR):
            saug_q[e // 2, e % 2, :] = (0.5 * sq[ai][rows]).astype(
                ml_dtypes.float8_e4m3
            )

        mask = np.full((128, 4096), NEG16, dtype=ml_dtypes.float8_e5m2)
        p = np.arange(128)[:, None]
        for m, t in enumerate(tiles):
            g = t * 128 + p  # global row index per partition
            # last 1024 keys of slot m's budget: [1024m, 1024(m+1))
            y = 1024 * m + np.arange(1024)[None, :]
            mask[:, m * 1024 : (m + 1) * 1024] = np.where(
                y <= g, 0.0, NEG16
            ).astype(ml_dtypes.float8_e5m2)
        in_maps.append(
            {
                "xqt": xqt,
                "xtp8": xtp8,
                "xp8": xp8,
                "wqk": wqkc,
                "wov": wov16,
                "mask": mask,
                "ident": np.eye(128, dtype=np.float16),
                "saug_q": np.ascontiguousarray(saug_q.reshape(3, 2 * RQ)),
                "saug_k": np.ascontiguousarray(saug_k.reshape(3, 2 * T)),
            }
        )
    return in_maps


def run(x, Wqk, Wov, **spmd_kwargs):
    """Full pipeline; returns (output [T, D] fp32, BassKernelResults)."""
    import time

    nc = _get_nc()
    in_maps = _prep_in_maps(x, Wqk, Wov)
    try:
        res = run_bass_kernel_spmd(
            nc, in_maps, core_ids=list(range(NCORES)), **spmd_kwargs
        )
    except Exception:
        # a prior crashed execution can leave a core transiently
        # unrecoverable; the runtime resets it — retry once
        time.sleep(10)
        res = run_bass_kernel_spmd(
            nc, in_maps, core_ids=list(range(NCORES)), **spmd_kwargs
        )
    out = np.empty((T, D), dtype=np.float32)
    for c in range(NCORES):
        co = res.results[c]["out"]
        for m, t in enumerate(_slot_tiles(c)):
            out[t * 128 : (t + 1) * 128] = co[m * 128 : (m + 1) * 128]
    return np.ascontiguousarray(out), res


def kernel(x, Wqk, Wov):
    out, _ = run(x, Wqk, Wov)
    return out

